# revision 1
# baseline (speedup 1.0000x reference)
"""1D row-parallel GAT across 8 NeuronCores.

Sharding (per spec hint): shard the N (node/row) dimension of x, adj,
scores and attention across the 8 cores; replicate per-head W/a and the
linear weights. Each core computes the full Wh = x @ W (cheap, replaces
the all-gather of Wh columns with replicated compute on the replicated
full x), then attention rows + aggregation + output linear for its
768-row shard. Outputs are gathered back to the full [6144, 128] array.
"""
import numpy as np

N = 6144
NFEAT = 512
NHID = 256
NHEADS = 4
DHEAD = NHID // NHEADS  # 64
NEMBED = 128
NEG_INF = -9e15
LRELU_ALPHA = 0.2
NCORES = 8
NS = N // NCORES  # 768 rows per core


def _build_jax_path():
    import jax
    import jax.numpy as jnp
    from functools import partial

    def shard_fwd(adj_s, x_s, x_full, W, a_src, a_dst, lin_w, lin_b):
        # Full Wh on every core: [H, N, D]
        Wh = jnp.einsum('nf,hfd->hnd', x_full, W)
        # Row-shard source term from the local x rows: [H, NS]
        Wh_s = jnp.einsum('nf,hfd->hnd', x_s, W)
        s = jnp.einsum('hnd,hd->hn', Wh_s, a_src)
        # Full destination term: [H, N]
        t = jnp.einsum('hnd,hd->hn', Wh, a_dst)
        e = s[:, :, None] + t[:, None, :]                     # [H, NS, N]
        e = jnp.where(e > 0, e, LRELU_ALPHA * e)              # leaky relu
        e = jnp.where(adj_s[None, :, :] > 0, e, NEG_INF)
        e = e - jax.lax.stop_gradient(jnp.max(e, axis=-1, keepdims=True))
        ex = jnp.exp(e)
        attn = ex / jnp.sum(ex, axis=-1, keepdims=True)
        h = jnp.einsum('hnm,hmd->hnd', attn, Wh)              # [H, NS, D]
        h = jnp.where(h > 0, h, jnp.expm1(h))                 # elu
        h = jnp.transpose(h, (1, 0, 2)).reshape(NS, NHID)
        out = h @ lin_w.T + lin_b
        out = jnp.where(out > 0, out, jnp.expm1(out))         # elu
        return out

    pmapped = jax.pmap(
        shard_fwd,
        in_axes=(0, 0, None, None, None, None, None, None),
        devices=jax.devices()[:NCORES],
    )
    return jax, jnp, pmapped, shard_fwd


def _numpy_fallback(x, adj, W, a_src, a_dst, lin_w, lin_b):
    Wh = np.einsum('nf,hfd->hnd', x, W)
    s = np.einsum('hnd,hd->hn', Wh, a_src)
    t = np.einsum('hnd,hd->hn', Wh, a_dst)
    e = s[:, :, None] + t[:, None, :]
    e = np.where(e > 0, e, LRELU_ALPHA * e)
    e = np.where(adj[None, :, :] > 0, e, NEG_INF)
    e -= e.max(axis=-1, keepdims=True)
    np.exp(e, out=e)
    e /= e.sum(axis=-1, keepdims=True)
    h = np.einsum('hnm,hmd->hnd', e, Wh)
    h = np.where(h > 0, h, np.expm1(h))
    h = np.transpose(h, (1, 0, 2)).reshape(N, NHID)
    out = h @ lin_w.T + lin_b
    return np.where(out > 0, out, np.expm1(out)).astype(np.float32)


def kernel(x, adj, W, a_src, a_dst, lin_w, lin_b):
    x = np.asarray(x, dtype=np.float32)
    adj = np.asarray(adj, dtype=np.int32)
    W = np.asarray(W, dtype=np.float32)
    a_src = np.asarray(a_src, dtype=np.float32)
    a_dst = np.asarray(a_dst, dtype=np.float32)
    lin_w = np.asarray(lin_w, dtype=np.float32)
    lin_b = np.asarray(lin_b, dtype=np.float32)
    try:
        jax, jnp, pmapped, shard_fwd = _build_jax_path()
        if len(jax.devices()) >= NCORES:
            adj_sh = adj.reshape(NCORES, NS, N)
            x_sh = x.reshape(NCORES, NS, NFEAT)
            out = pmapped(adj_sh, x_sh, x, W, a_src, a_dst, lin_w, lin_b)
            return np.asarray(out, dtype=np.float32).reshape(N, NEMBED)
        # single-device jax fallback
        outs = []
        fwd = jax.jit(shard_fwd)
        for i in range(NCORES):
            outs.append(np.asarray(fwd(
                adj[i * NS:(i + 1) * NS], x[i * NS:(i + 1) * NS],
                x, W, a_src, a_dst, lin_w, lin_b)))
        return np.concatenate(outs, 0).astype(np.float32)
    except Exception:
        return _numpy_fallback(x, adj, W, a_src, a_dst, lin_w, lin_b)

